# revision 27
# baseline (speedup 1.0000x reference)
"""Trainium2 Bass kernel for DynConvLayer (512x512, C=64, K=3, dil=2).

out = where(sd, gelu(conv2(rpad(x_ori))), gelu(dwconv3(rpad(x)))) + x
  x_ori = where(md, gelu(conv1(rpad(x))), x)
  md = 5x5-binary-dilate(mask), sd = mask>0.5, rpad = reflect-pad-2

Sharding: H split across 8 cores (64 rows each, halo 4), W split into 4
panels of 128 cols per core (SBUF capacity). Convs run on the tensor
engine as per-tap [C_in, C_out] matmuls accumulated in PSUM over a
flattened (row*136+col) pixel stream.

conv1+conv3 (fused): a +2-row-shifted copy of x in SBUF partitions
64..127 lets one K=128 matmul cover two taps; the depthwise conv3 rides
in PSUM partitions 64..127 as diagonal weight columns. gelu(conv1) is
written by ACT directly into x_ori, then a predicated copy pulls x back
where ~md (no separate seed DMA).

conv2 (pixel-paired): output rows are processed in blocks of 4 as two
row-pairs (4b,4b+1 | 4b+2,4b+3) living in PSUM partitions 0..63/64..127
of one [128, 272] accumulation. The 3 "diag" matmuls are fully dense
128x128 (each covers 4 tap-applications via the +2-shifted B-half); the
3 "corner" matmuls use an auxiliary +6-shifted pairing tile (X6). This
halves conv2's tensor-engine time versus per-tap M=64 matmuls.

Matmul inputs are bf16 (fp32 PSUM accumulate); the residual +x is added
on the host in fp32. Reflection halos are handled by host padding plus
on-chip strided fix-up copies and per-core edge-mask blends, keeping the
program SPMD-uniform across all 8 cores. DMA traffic is split between
the SP and GpSimd queues.
"""

import os
import sys

import numpy as np

for _p in ("/opt/trn_rl_repo", "/opt/pypackages"):
    if _p not in sys.path:
        sys.path.insert(0, _p)

import concourse.bass as bass
import concourse.bacc as bacc
import concourse.mybir as mybir
from concourse.tile import TileContext
from concourse.bass_utils import run_bass_kernel_spmd

F32 = mybir.dt.float32
BF16 = mybir.dt.bfloat16
U8 = mybir.dt.uint8
AF = mybir.ActivationFunctionType

C = 64
H = W = 512
NCORES = 8
RPC = H // NCORES          # 64 output rows per core
NP = 4                     # W panels per core
PCOL = W // NP             # 128 output cols per panel
PW = PCOL + 8              # 136 slab width (cols -4..132 rel panel)
XROWS = 72                 # x slab rows (-4..68 rel core block)
OROWS = RPC + 4            # 68 rows of x_ori stream (-2..65)
S13 = OROWS * PW           # 9248 conv13 stream length
S2 = RPC * PW              # 8704 conv2 / output stream length
MX = 2                     # front margin of x tile (negative tap offsets)
MO = 2                     # front margin of x_ori tile
MOX = 2                    # front margin of X6 tile
XF = MX + S13 + 4 * PW + 8     # x tile free size
XOF = MO + S2 + 4 * PW + 8     # x_ori tile free size
X6A = 64 * PW              # X6 A-half extent (x_ori rows -2..61)
X6B = 62 * PW              # X6 B-half extent (s+6PW = rows 4..65)
XF6 = MOX + X6A + 8        # X6 tile free size
ROWOFF13 = 2 * PW          # conv13 stream -> x tile row offset
CHUNK = 512
NB = RPC // 4              # 16 conv2 blocks of 4 output rows

_CACHE = {}


def _chunks(total):
    out = []
    off = 0
    while off < total:
        n = min(CHUNK, total - off)
        out.append((off, n))
        off += n
    return out


def _build_program(act=None):
    act = AF.Gelu if act is None else act
    key = ("nc", str(act))
    if key in _CACHE:
        return _CACHE[key]
    nc = bacc.Bacc("TRN2", target_bir_lowering=False, debug=False)

    x_in = nc.declare_dram_parameter("x_in", [NP, 128, XROWS * PW], BF16, isOutput=False)
    md_in = nc.declare_dram_parameter("md_in", [NP, 64, S13], U8, isOutput=False)
    sd_in = nc.declare_dram_parameter("sd_in", [NP, 128, NB * 256], U8, isOutput=False)
    w13p_in = nc.declare_dram_parameter("w13p", [128, 3 * 128], BF16, isOutput=False)
    w13s_in = nc.declare_dram_parameter("w13s", [64, 3 * 128], BF16, isOutput=False)
    w2d_in = nc.declare_dram_parameter("w2d", [128, 3 * 128], BF16, isOutput=False)
    w2c_in = nc.declare_dram_parameter("w2c", [128, 3 * 128], BF16, isOutput=False)
    b1_in = nc.declare_dram_parameter("b1d", [64, 1], F32, isOutput=False)
    b3_in = nc.declare_dram_parameter("b3d", [64, 1], F32, isOutput=False)
    etop_in = nc.declare_dram_parameter("etopm", [64, PW], U8, isOutput=False)
    ebot_in = nc.declare_dram_parameter("ebotm", [64, PW], U8, isOutput=False)
    b2d_in = nc.declare_dram_parameter("b2dup", [128, 1], F32, isOutput=False)
    # bf16 output, rows ordered [even-pairs | odd-pairs]; host reorders
    out_d = nc.declare_dram_parameter("out", [NP, 64, RPC * PCOL], BF16, isOutput=True)

    ch13 = _chunks(S13)

    with TileContext(nc) as tc:
        with (
            tc.tile_pool(name="const", bufs=1) as cpool,
            tc.tile_pool(name="xp", bufs=2) as xpool,
            tc.tile_pool(name="xop", bufs=2) as xopool,
            tc.tile_pool(name="x6p", bufs=2) as x6pool,
            tc.tile_pool(name="mp", bufs=1) as mpool,
            tc.tile_pool(name="op", bufs=2) as opool,
            tc.tile_pool(name="ps13", bufs=8, space="PSUM") as ps13pool,
        ):
            w13pt = cpool.tile([128, 3 * 128], BF16, name="w13pt")
            w13st = cpool.tile([64, 3 * 128], BF16, name="w13st")
            w2dt = cpool.tile([128, 3 * 128], BF16, name="w2dt")
            w2ct = cpool.tile([128, 3 * 128], BF16, name="w2ct")
            b1t = cpool.tile([64, 1], F32, name="b1t")
            b3t = cpool.tile([64, 1], F32, name="b3t")
            b2t = cpool.tile([128, 1], F32, name="b2t")
            etopt = cpool.tile([64, PW], U8, name="etopt")
            ebott = cpool.tile([64, PW], U8, name="ebott")
            for t, d in (
                (w13pt, w13p_in), (w13st, w13s_in), (w2dt, w2d_in),
                (w2ct, w2c_in), (b1t, b1_in), (b3t, b3_in), (b2t, b2d_in),
                (etopt, etop_in), (ebott, ebot_in),
            ):
                nc.sync.dma_start(out=t[:, :], in_=d.ap())

            def conv2_block(p, b, xori, x6t, sdt, outt, g3t):
                ps = ps13pool.tile([128, CHUNK], F32, name=f"ps2_{p}_{b}", tag="ps13")
                for kc in range(3):
                    dc = 2 * (kc - 1)
                    nc.tensor.matmul(
                        ps[0:128, 0:272],
                        w2dt[0:128, 128 * kc: 128 * (kc + 1)],
                        xori[0:128, MO + (4 * b + 2) * PW + dc: MO + (4 * b + 2) * PW + dc + 272],
                        start=(kc == 0), stop=False,
                    )
                for kc in range(3):
                    dc = 2 * (kc - 1)
                    nc.tensor.matmul(
                        ps[0:128, 0:272],
                        w2ct[0:128, 128 * kc: 128 * (kc + 1)],
                        x6t[0:128, MOX + 4 * b * PW + dc: MOX + 4 * b * PW + dc + 272],
                        start=False, stop=(kc == 2),
                    )
                # gelu(conv2 + b2) for all 4 rows in one ACT, packed into
                # the interleaved output tile; then pull gelu3 where ~sd.
                ob = outt[0:128, 256 * b: 256 * (b + 1)].rearrange(
                    "a (r c) -> a r c", c=128)
                nc.scalar.activation(
                    ob,
                    ps[0:128, 0:272].rearrange(
                        "a (r c) -> a r c", c=PW)[:, :, 4:132],
                    act, bias=b2t[0:128, 0:1],
                )
                sb = sdt[0:128, 256 * b: 256 * (b + 1)].rearrange(
                    "a (r c) -> a r c", c=128)
                g3v = g3t[:, 0:S2].rearrange("a (r c) -> a r c", c=PW)
                nc.vector.copy_predicated(
                    ob[0:64], sb[0:64],
                    g3v[0:64, 4 * b: 4 * b + 2, 4:132],
                )
                nc.vector.copy_predicated(
                    ob[64:128], sb[64:128],
                    g3v[64:128, 4 * b + 2: 4 * b + 4, 4:132],
                )
                if b % 4 == 3:
                    g = b // 4
                    eng = nc.sync if p == NP - 1 else nc.gpsimd
                    eng.dma_start(
                        out=out_d.ap()[p, 0:64, 1024 * g: 1024 * (g + 1)],
                        in_=outt[0:64, 1024 * g: 1024 * (g + 1)],
                    )
                    eng.dma_start(
                        out=out_d.ap()[p, 0:64, 4096 + 1024 * g: 4096 + 1024 * (g + 1)],
                        in_=outt[64:128, 1024 * g: 1024 * (g + 1)],
                    )

            pending = []
            for p in range(NP):
                xt = xpool.tile([128, XF], BF16, name=f"xt{p}", tag="xt")
                xori = xopool.tile([128, XOF], BF16, name=f"xori{p}", tag="xori")
                x6t = x6pool.tile([128, XF6], BF16, name=f"x6_{p}", tag="x6")
                mdt = mpool.tile([64, S13], U8, name=f"mdt{p}", tag="mdt")
                sdt = mpool.tile([128, NB * 256], U8, name=f"sdt{p}", tag="sdt")
                outt = opool.tile([128, NB * 256], BF16, name=f"outt{p}", tag="outt")
                g3t = opool.tile([128, S2], BF16, name=f"g3_{p}", tag="g3")

                # margins: read by garbage output positions, never used
                nc.vector.memset(xt[0:128, 0:MX], 0.0)
                nc.vector.memset(xt[0:128, MX + XROWS * PW: XF], 0.0)
                nc.vector.memset(xori[0:128, 0:MO], 0.0)
                nc.vector.memset(xori[0:128, MO + S13: XOF], 0.0)
                nc.vector.memset(xori[64:128, MO + S13 - 2 * PW: MO + S13], 0.0)
                nc.vector.memset(x6t[0:128, 0:MOX], 0.0)
                nc.vector.memset(x6t[0:128, MOX + X6A: XF6], 0.0)
                nc.vector.memset(x6t[64:128, MOX + X6B: MOX + X6A], 0.0)

                # input DMAs (x in row-bands, alternating queues, so compute
                # can start early)
                for bi, (r0, r1) in enumerate(((0, 10), (10, 24), (24, 48), (48, XROWS))):
                    eng = nc.gpsimd if bi % 2 == 0 else nc.sync
                    eng.dma_start(
                        out=xt[0:128, MX + r0 * PW: MX + r1 * PW],
                        in_=x_in.ap()[p, :, r0 * PW: r1 * PW],
                    )
                nc.sync.dma_start(out=mdt[0:64, 0:S13], in_=md_in.ap()[p])
                nc.sync.dma_start(out=sdt[0:128, :], in_=sd_in.ap()[p])

                # ---- conv1 + conv3 fused; per-group epilogue releases
                # fixups/blends/B-copy incrementally so conv2 starts gap-free
                xov = xori[0:64, MO: MO + S13].rearrange("a (r c) -> a r c", c=PW)
                done_prev = 0
                b_prev = 0
                a6_prev = 0
                b6_prev = 0
                g3_prev = 0
                for gi in range(0, len(ch13), 8):
                    grp = ch13[gi: gi + 8]
                    pst = [
                        ps13pool.tile([128, CHUNK], F32, name=f"ps13_{p}_{gi + k}", tag="ps13")
                        for k in range(len(grp))
                    ]
                    for wdx in range(6):
                        if wdx < 3:  # tap pair (-2,dc)+(0,dc), K=128
                            dc = 2 * (wdx - 1)
                            lhs = w13pt[0:128, 128 * wdx: 128 * (wdx + 1)]
                            for k, (o, n) in enumerate(grp):
                                nc.tensor.matmul(
                                    pst[k][0:128, 0:n],
                                    lhs,
                                    xt[0:128, MX + o + dc: MX + o + dc + n],
                                    start=(wdx == 0), stop=(wdx == 5),
                                )
                        else:  # single tap (2,dc), K=64
                            dc = 2 * (wdx - 4)
                            lhs = w13st[0:64, 128 * (wdx - 3): 128 * (wdx - 2)]
                            for k, (o, n) in enumerate(grp):
                                off = MX + o + 4 * PW + dc
                                nc.tensor.matmul(
                                    pst[k][0:128, 0:n],
                                    lhs,
                                    xt[0:64, off: off + n],
                                    start=(wdx == 0), stop=(wdx == 5),
                                )
                    for k, (o, n) in enumerate(grp):
                        # gelu(conv1) straight into x_ori, then pull x where ~md
                        nc.scalar.activation(
                            xori[0:64, MO + o: MO + o + n], pst[k][0:64, 0:n],
                            act, bias=b1t[0:64, 0:1],
                        )
                        nc.vector.copy_predicated(
                            xori[0:64, MO + o: MO + o + n],
                            mdt[0:64, o: o + n],
                            xt[0:64, MX + o + 2 * PW: MX + o + 2 * PW + n],
                        )
                        # gelu(conv3) into g3 A-half (cross-partition ACT:
                        # PSUM parts 64..127 -> SBUF parts 0..63)
                        qa = max(o, ROWOFF13)
                        qb = min(o + n, ROWOFF13 + S2)
                        if qa < qb:
                            nc.scalar.activation(
                                g3t[0:64, qa - ROWOFF13: qb - ROWOFF13],
                                pst[k][64:128, qa - o: qb - o],
                                act, bias=b3t[0:64, 0:1],
                            )
                    # -- group epilogue over fully-predicated rows
                    o_end = grp[-1][0] + grp[-1][1]
                    last = o_end >= S13
                    done = OROWS if last else o_end // PW
                    if done > done_prev:
                        if p == 0:
                            for dst, src in ((2, 6), (3, 5)):
                                nc.vector.tensor_copy(
                                    xov[:, done_prev:done, dst: dst + 1],
                                    xov[:, done_prev:done, src: src + 1],
                                )
                        if p == NP - 1:
                            for dst, src in ((132, 130), (133, 129)):
                                nc.vector.tensor_copy(
                                    xov[:, done_prev:done, dst: dst + 1],
                                    xov[:, done_prev:done, src: src + 1],
                                )
                    if done_prev < 5 <= done:
                        # top reflect blend (rows -2,-1 <- 2,1), cores 0/7 only
                        for dst, src in ((0, 4), (1, 3)):
                            nc.vector.copy_predicated(
                                xori[0:64, MO + dst * PW: MO + (dst + 1) * PW],
                                etopt[0:64, 0:PW],
                                xori[0:64, MO + src * PW: MO + (src + 1) * PW],
                            )
                    if last:
                        for dst, src in ((OROWS - 2, OROWS - 4), (OROWS - 1, OROWS - 5)):
                            nc.vector.copy_predicated(
                                xori[0:64, MO + dst * PW: MO + (dst + 1) * PW],
                                ebott[0:64, 0:PW],
                                xori[0:64, MO + src * PW: MO + (src + 1) * PW],
                            )
                    # B-half pieces: B row r := x_ori row r+2 (rows 64,65 need
                    # the bottom blend, so they wait for the last group)
                    b_hi = OROWS - 2 if last else min(done - 2, OROWS - 4)
                    if b_hi > b_prev:
                        nc.sync.dma_start(
                            out=xori[64:128, MO + b_prev * PW: MO + b_hi * PW],
                            in_=xori[0:64, MO + (b_prev + 2) * PW: MO + (b_hi + 2) * PW],
                        )
                    b_prev = max(b_prev, b_hi)
                    # g3 B-half duplicate (keeps conv2 odd-half blends
                    # partition-aligned)
                    g3_hi = min(o_end, ROWOFF13 + S2) - ROWOFF13
                    if g3_hi > g3_prev:
                        nc.sync.dma_start(
                            out=g3t[64:128, g3_prev:g3_hi],
                            in_=g3t[0:64, g3_prev:g3_hi],
                        )
                        g3_prev = g3_hi
                    # X6 pieces (conv2 corner pairing): A-half = x_ori stream
                    # rows 0..63 (x_ori rows -2..61), B-half = A shifted +6 rows.
                    # Gate on top blend (done>=5) and bottom blend (last).
                    if done >= 5:
                        a6_hi = 64 if last else max(0, min(done - 2, 64))
                        if a6_hi > a6_prev:
                            nc.gpsimd.dma_start(
                                out=x6t[0:64, MOX + a6_prev * PW: MOX + a6_hi * PW],
                                in_=xori[0:64, MO + a6_prev * PW: MO + a6_hi * PW],
                            )
                            a6_prev = a6_hi
                        b6_hi = 62 if last else max(0, min(done - 8, 62))
                        if b6_hi > b6_prev:
                            nc.gpsimd.dma_start(
                                out=x6t[64:128, MOX + b6_prev * PW: MOX + b6_hi * PW],
                                in_=xori[0:64, MO + (b6_prev + 6) * PW: MO + (b6_hi + 6) * PW],
                            )
                            b6_prev = b6_hi
                    done_prev = done
                    # pipeline: emit the previous panel's deferred conv2 tail
                    # after this panel's first matmul group, so those blocks'
                    # X6/B-half dependencies have cleared by the time the
                    # tensor engine reaches them
                    if gi == 0 and pending:
                        for th in pending:
                            th()
                        pending = []

                # ---- conv2 on x_ori: pixel-paired blocks of 4 output rows.
                # PSUM parts 0..63 = out rows (4b,4b+1), 64..127 = (4b+2,4b+3).
                for b in range(12):
                    conv2_block(p, b, xori, x6t, sdt, outt, g3t)
                tail = [
                    (lambda p=p, b=b, xori=xori, x6t=x6t, sdt=sdt, outt=outt, g3t=g3t:
                     conv2_block(p, b, xori, x6t, sdt, outt, g3t))
                    for b in range(12, NB)
                ]
                if p < NP - 1:
                    pending = tail
                else:
                    for th in tail:
                        th()

    nc.compile()
    _CACHE[key] = nc
    return nc


def _pack_weights(w1, w2, w3, b1, b2, b3):
    w13p = np.zeros((128, 3, 128), np.float32)
    w13s = np.zeros((64, 3, 128), np.float32)
    w2d = np.zeros((128, 3, 128), np.float32)
    w2c = np.zeros((128, 3, 128), np.float32)
    di = np.arange(64)
    for k in range(3):
        w13p[0:64, k, 0:64] = w1[:, :, 0, k].T
        w13p[64:128, k, 0:64] = w1[:, :, 1, k].T
        w13p[di, k, 64 + di] = w3[:, 0, 0, k]
        w13p[64 + di, k, 64 + di] = w3[:, 0, 1, k]

        w13s[0:64, k, 0:64] = w1[:, :, 2, k].T
        w13s[di, k, 64 + di] = w3[:, 0, 2, k]

        # conv2 pixel-pair: dense diag matmul covers 4 tap-applications
        w2d[0:64, k, 0:64] = w2[:, :, 1, k].T     # x(4b..) -> out(4b..)
        w2d[0:64, k, 64:128] = w2[:, :, 0, k].T   # x(4b..) -> out(4b+2..)
        w2d[64:128, k, 0:64] = w2[:, :, 2, k].T   # x(4b+2..) -> out(4b..)
        w2d[64:128, k, 64:128] = w2[:, :, 1, k].T
        # corner matmul via X6 (+6-shift pairing)
        w2c[0:64, k, 0:64] = w2[:, :, 0, k].T     # x(4b-2..) -> out(4b..)
        w2c[64:128, k, 64:128] = w2[:, :, 2, k].T  # x(4b+4..) -> out(4b+2..)
    return (
        np.ascontiguousarray(w13p.reshape(128, 384)),
        np.ascontiguousarray(w13s.reshape(64, 384)),
        np.ascontiguousarray(w2d.reshape(128, 384)),
        np.ascontiguousarray(w2c.reshape(128, 384)),
        b1.reshape(64, 1).astype(np.float32),
        b3.reshape(64, 1).astype(np.float32),
        np.concatenate([b2, b2]).reshape(128, 1).astype(np.float32),
    )


def _dilate5(m):
    # 5x5 binary dilation, SAME/zero-pad semantics (max-pool)
    hh, ww = m.shape
    mp = np.pad(m, 2)
    a = np.maximum.reduce([mp[k: k + hh] for k in range(5)])      # [hh, ww+4]
    return np.maximum.reduce([a[:, k: k + ww] for k in range(5)])  # [hh, ww]


def make_in_maps(x, mask, w1, b1, w2, b2, w3, b3):
    import ml_dtypes
    BF = ml_dtypes.bfloat16
    x = np.asarray(x, np.float32)
    mask = np.asarray(mask, np.float32)

    w13p, w13s, w2d, w2c, b1p, b3p, b2dp = _pack_weights(
        np.asarray(w1, np.float32), np.asarray(w2, np.float32),
        np.asarray(w3, np.float32), np.asarray(b1, np.float32),
        np.asarray(b2, np.float32), np.asarray(b3, np.float32))
    w13p = w13p.astype(BF); w13s = w13s.astype(BF)
    w2d = w2d.astype(BF); w2c = w2c.astype(BF)

    xp32 = np.pad(x[0], ((0, 0), (4, 6), (4, 4)), mode="reflect")  # [64,522,520]
    xp = xp32.astype(BF)
    m = mask[0, 0]
    # inverted dilated mask: blend pulls x where ~md
    mdi = 1 - (_dilate5(m) > 0.5).astype(np.uint8)
    mdp = np.pad(mdi, ((2, 2), (4, 4)), mode="edge")   # [516,520]
    sdi = 1 - (m > 0.5).astype(np.uint8)               # inverted: pull gelu3

    ones = np.ones((64, PW), np.uint8)
    zeros = np.zeros((64, PW), np.uint8)

    in_maps = []
    for i in range(NCORES):
        r0 = RPC * i
        xc = np.empty((NP, 128, XROWS, PW), BF)
        mdc = np.empty((NP, 64, OROWS, PW), np.uint8)
        sdc = np.empty((NP, 128, NB, 2, PCOL), np.uint8)
        for p in range(NP):
            c0 = PCOL * p
            xc[p, 0:64] = xp[:, r0: r0 + XROWS, c0: c0 + PW]
            xc[p, 64:128] = xp[:, r0 + 2: r0 + 2 + XROWS, c0: c0 + PW]
            mdc[p] = np.broadcast_to(
                mdp[r0: r0 + OROWS, c0: c0 + PW], (64, OROWS, PW))
            # packed interleaved ~sd: parts 0-63 = rows (4b,4b+1),
            # parts 64-127 = rows (4b+2,4b+3)
            sdi4 = sdi[r0: r0 + RPC, c0: c0 + PCOL].reshape(NB, 2, 2, PCOL)
            sdc[p, 0:64] = np.broadcast_to(sdi4[:, 0], (64, NB, 2, PCOL))
            sdc[p, 64:128] = np.broadcast_to(sdi4[:, 1], (64, NB, 2, PCOL))
        xc = np.ascontiguousarray(xc).reshape(NP, 128, XROWS * PW)
        mdc = np.ascontiguousarray(mdc).reshape(NP, 64, S13)
        sdc = np.ascontiguousarray(sdc).reshape(NP, 128, NB * 256)
        in_maps.append({
            "x_in": xc, "md_in": mdc, "sd_in": sdc,
            "w13p": w13p, "w13s": w13s, "w2d": w2d, "w2c": w2c,
            "b1d": b1p, "b3d": b3p, "b2dup": b2dp,
            "etopm": ones if i == 0 else zeros,
            "ebotm": ones if i == NCORES - 1 else zeros,
        })

    return in_maps


def kernel(x, mask, w1, b1, w2, b2, w3, b3):
    nc = _build_program()
    in_maps = make_in_maps(x, mask, w1, b1, w2, b2, w3, b3)
    global _last_in_maps
    _last_in_maps = in_maps
    res = run_bass_kernel_spmd(nc, in_maps, list(range(NCORES)))
    out = np.empty((1, C, H, W), np.float32)
    for i in range(NCORES):
        o = np.asarray(res.results[i]["out"], np.float32)  # [NP, 64, 8192] bf16
        # rows ordered [even-pairs | odd-pairs]: (half, b, rr) -> 4b+2*half+rr
        o = o.reshape(NP, C, 2, NB, 2, PCOL).transpose(1, 3, 2, 4, 0, 5)
        out[0, :, RPC * i: RPC * (i + 1), :] = o.reshape(C, RPC, W)
    out += np.asarray(x, np.float32).reshape(1, C, H, W)
    return out
